# revision 10
# baseline (speedup 1.0000x reference)
"""Trainium2 Bass kernel for CustomAttentionWithPE (bf16 matmul pipeline).

Reference computation (B=2, S=2048, H=16, Dh=64, D=1024):
    qkv = hs @ W_qkv + b_qkv ; split to q,k,v per head
    q,k = RoPE(q), RoPE(k)
    out = softmax(q k^T / 8) v   (no mask)
    return concat_heads(out) @ W_o + b_o

Sharding: 8 cores -> (batch b = core//4, head-quad g = core%4, heads 4g..4g+3).
Each core computes partial = attn(heads of g, batch b) @ W_o[rows of g]
for its batch; host sums the 4 partials per batch and adds the bias terms
(b_o and the V-bias contribution b_v @ W_o; softmax rows sum to 1 so the
V bias contributes exactly b_v @ W_o per token).

All matmuls run in bf16 (4x PE throughput vs fp32; fp32 PSUM accumulate).
Inputs are converted to bf16 on the host; output returns as bf16 partials
summed in fp32 on the host. Measured end-to-end gate error ~7e-3 vs the
2e-2 budget.

Device pipeline per core:
  x^T resident in SBUF [128, 8, 2048] bf16.
  QT/KT computed transposed ([dh-rows, token-cols]); RoPE on fp32 raw
  copies (DVE), result stored bf16. V in natural [token, vcol] layout
  with a ones column so PV also produces the softmax denominator Z.
  scores^T[k,q] per head via PE (contract dh=64); head pairs packed at
  PE row offsets 0/64. exp on ScalarE in [128, 2048] batches (both
  heads of a pair in one instruction; scale=0.125 folds the 1/sqrt(dh)).
  PV accumulates over 16 k-tiles into PSUM [65, 512]; normalization
  multiplies by 1/Z (reciprocal_approx_fast) broadcast across partitions
  via a rank-1 PE matmul. Output projection contracts the 256 local
  head-dims in 2 chunks of 128.
"""

import math
from contextlib import ExitStack

import numpy as np
import ml_dtypes

import concourse.bass as bass
import concourse.mybir as mybir
import concourse.tile as tile
from concourse.bass_utils import run_bass_kernel_spmd

F32 = mybir.dt.float32
BF16 = mybir.dt.bfloat16
AF = mybir.ActivationFunctionType

B, S, D = 2, 2048, 1024
NH, HD = 16, 64
ROPE_BASE = 10000.0
N_CORES = 8
HPC = 4  # heads per core
DLOC = HPC * HD  # 256 local head dims per core


def _split_sync_waits(nc, maxw=1):
    """This container's walrus rejects >1-2 SyncWaits per instruction
    ("Too many sync wait commands"). Move excess waits onto NoOps."""
    for f in nc.m.functions:
        for blk in f.blocks:
            new_instructions = []
            for ins in blk.instructions:
                si = getattr(ins, "sync_info", None)
                if si is not None and si.on_wait and len(si.on_wait) > maxw:
                    waits = list(si.on_wait)
                    extra, keep = waits[:-maxw], waits[-maxw:]
                    si.on_wait = keep
                    for i in range(0, len(extra), maxw):
                        nop = mybir.InstNoOp(
                            name=nc.get_next_instruction_name(),
                            engine=ins.engine,
                            sync_info=mybir.SyncInfo(
                                on_wait=extra[i : i + maxw], on_update=[]
                            ),
                        )
                        nc.register_instruction(nop, overwrite=True)
                        new_instructions.append(nop)
                new_instructions.append(ins)
            blk.instructions[:] = new_instructions


def build_attention_nc(seq=S, add_qk_bias=False):
    """One SPMD program; per-core data differs only through inputs."""
    nc = bass.Bass()
    NS = seq // 1024  # phase-1 token stripes
    KT = seq // 128  # k tiles
    NT = seq // 512  # q stripes
    QG = 2  # k-tiles per exp group
    NCH = D // 128  # contraction chunks over d_model

    xT = nc.dram_tensor("xT", [D, seq], BF16, kind="ExternalInput")
    wq = nc.dram_tensor("wq", [D, DLOC], BF16, kind="ExternalInput")
    wk = nc.dram_tensor("wk", [D, DLOC], BF16, kind="ExternalInput")
    wv = nc.dram_tensor("wv", [D, DLOC], BF16, kind="ExternalInput")
    wo = nc.dram_tensor("wo", [DLOC, D], BF16, kind="ExternalInput")
    cosT = nc.dram_tensor("cosT", [HD, seq], F32, kind="ExternalInput")
    sinT = nc.dram_tensor("sinT", [HD, seq], F32, kind="ExternalInput")
    bqk = nc.dram_tensor("bqk", [2, DLOC], F32, kind="ExternalInput")
    out = nc.dram_tensor("out", [seq, D], BF16, kind="ExternalOutput")

    with tile.TileContext(nc) as tc, ExitStack() as ctx:
        consts = ctx.enter_context(tc.tile_pool(name="consts", bufs=1))
        # x^T resident: row d = c*128 + p, col = token
        x_sb = consts.tile([128, NCH, seq], BF16)
        nc.sync.dma_start(out=x_sb, in_=xT.rearrange("(c p) s -> p c s", p=128))
        wq_sb = consts.tile([128, NCH, DLOC], BF16)
        nc.sync.dma_start(out=wq_sb, in_=wq.rearrange("(c p) m -> p c m", p=128))
        wk_sb = consts.tile([128, NCH, DLOC], BF16)
        nc.sync.dma_start(out=wk_sb, in_=wk.rearrange("(c p) m -> p c m", p=128))
        wv_sb = consts.tile([128, NCH, DLOC], BF16)
        nc.sync.dma_start(out=wv_sb, in_=wv.rearrange("(c p) m -> p c m", p=128))
        wo_sb = consts.tile([128, 2, D], BF16)
        nc.sync.dma_start(out=wo_sb, in_=wo.rearrange("(c p) m -> p c m", p=128))
        # cos/sin rows duplicated for the two heads of a pair (fp32 RoPE)
        cs_sb = consts.tile([128, seq], F32)
        nc.sync.dma_start(out=cs_sb[0:HD, :], in_=cosT[:])
        nc.sync.dma_start(out=cs_sb[HD:128, :], in_=cosT[:])
        sn_sb = consts.tile([128, seq], F32)
        nc.sync.dma_start(out=sn_sb[0:HD, :], in_=sinT[:])
        nc.sync.dma_start(out=sn_sb[HD:128, :], in_=sinT[:])
        ones_sb = consts.tile([128, HD], BF16)
        nc.vector.memset(ones_sb, 1.0)
        if add_qk_bias:
            bqk_sb = consts.tile([128, 2, 2], F32)
            nc.sync.dma_start(
                out=bqk_sb, in_=bqk.rearrange("b (h p) -> p b h", p=128)
            )

        # long-lived activation tensors
        acts = ctx.enter_context(tc.tile_pool(name="acts", bufs=1))
        qtr = acts.tile([128, 2, seq], BF16)  # RoPE'd Q^T, head pairs
        ktr = acts.tile([128, 2, seq], BF16)
        v_sb = acts.tile([128, KT, HPC, HD + 2], BF16)  # V natural + ones col
        att = acts.tile([128, 2, seq], BF16)  # normalized attn out ^T
        qt_raw = acts.tile([128, 2, seq], F32)
        kt_raw = acts.tile([128, 2, seq], F32)
        nc.vector.memset(v_sb[:, :, :, HD : HD + 1], 1.0)

        # ---------------- phase 1: QKV projection + RoPE -------------
        with ExitStack() as p1:
            ps_qk = p1.enter_context(
                tc.tile_pool(name="ps_qk", bufs=2, space="PSUM")
            )
            ps_v = p1.enter_context(
                tc.tile_pool(name="ps_v", bufs=2, space="PSUM")
            )
            rope_tmp = p1.enter_context(tc.tile_pool(name="ropetmp", bufs=2))

            for ns in range(NS):
                toks = slice(ns * 1024, ns * 1024 + 1024)
                for hp in range(2):
                    for dst, w in ((qt_raw, wq_sb), (kt_raw, wk_sb)):
                        ps = ps_qk.tile([128, 1024], F32, tag="qk", name="psqk")
                        for half in range(2):
                            hs_ = slice(
                                ns * 1024 + half * 512, ns * 1024 + half * 512 + 512
                            )
                            for c in range(NCH):
                                nc.tensor.matmul(
                                    ps[:, half * 512 : half * 512 + 512],
                                    w[:, c, hp * 128 : hp * 128 + 128],
                                    x_sb[:, c, hs_],
                                    start=(c == 0),
                                    stop=(c == NCH - 1),
                                )
                        nc.vector.tensor_copy(dst[:, hp, toks], ps)
                # V natural: out [128 tokens, 256 vcols]
                for tt in range(8):
                    kt_idx = ns * 8 + tt
                    ts = slice(kt_idx * 128, kt_idx * 128 + 128)
                    ps = ps_v.tile([128, HPC, HD], F32, tag="v", name="psv")
                    for c in range(NCH):
                        nc.tensor.matmul(
                            ps,
                            x_sb[:, c, ts],
                            wv_sb[:, c, :],
                            start=(c == 0),
                            stop=(c == NCH - 1),
                        )
                    nc.vector.tensor_copy(v_sb[:, kt_idx, :, 0:HD], ps)

                if add_qk_bias:
                    for hp in range(2):
                        nc.vector.tensor_scalar_add(
                            qt_raw[:, hp, toks],
                            qt_raw[:, hp, toks],
                            bqk_sb[:, 0, hp : hp + 1],
                        )
                        nc.vector.tensor_scalar_add(
                            kt_raw[:, hp, toks],
                            kt_raw[:, hp, toks],
                            bqk_sb[:, 1, hp : hp + 1],
                        )

                # RoPE for this stripe: dst = raw*cos + rot(raw)*sin
                # rot rows (per 64-block): [0:32] = -raw[32:64], [32:64] = +raw[0:32]
                for raw, dst in ((qt_raw, qtr), (kt_raw, ktr)):
                    for hp in range(2):
                        for half in range(2):
                            cs = slice(
                                ns * 1024 + half * 512, ns * 1024 + half * 512 + 512
                            )
                            rot = rope_tmp.tile([128, 512], F32, tag="rot")
                            for base in (0, 64):
                                nc.vector.tensor_scalar_mul(
                                    rot[base : base + 32, :],
                                    raw[base + 32 : base + 64, hp, cs],
                                    -1.0,
                                )
                                nc.vector.tensor_copy(
                                    rot[base + 32 : base + 64, :],
                                    raw[base : base + 32, hp, cs],
                                )
                            tmp = rope_tmp.tile([128, 512], F32, tag="tmp")
                            nc.vector.tensor_mul(tmp, raw[:, hp, cs], cs_sb[:, cs])
                            nc.vector.tensor_mul(rot, rot, sn_sb[:, cs])
                            nc.vector.tensor_add(dst[:, hp, cs], tmp, rot)

        # ---------------- phase 2: attention + output projection -----
        with ExitStack() as p2:
            ps_sc = p2.enter_context(
                tc.tile_pool(name="ps_sc", bufs=2, space="PSUM")
            )
            ps_pv = p2.enter_context(
                tc.tile_pool(name="ps_pv", bufs=2, space="PSUM")
            )
            ps_zb = p2.enter_context(
                tc.tile_pool(name="ps_zb", bufs=1, space="PSUM")
            )
            ps_wo = p2.enter_context(
                tc.tile_pool(name="ps_wo", bufs=1, space="PSUM")
            )
            ptpool = p2.enter_context(tc.tile_pool(name="ptpool", bufs=3))
            npool = p2.enter_context(tc.tile_pool(name="norm", bufs=4))
            opool = p2.enter_context(tc.tile_pool(name="ostage", bufs=2))

            for qt in range(NT):
                qs = slice(qt * 512, qt * 512 + 512)
                for hp in range(2):
                    pv = [
                        ps_pv.tile([128, 512], F32, tag="pv", name="pv0"),
                        ps_pv.tile([128, 512], F32, tag="pv", name="pv1"),
                    ]
                    for g in range(KT // QG):
                        sc = [
                            ps_sc.tile([128, 1024], F32, tag="sc", name="sc0"),
                            ps_sc.tile([128, 1024], F32, tag="sc", name="sc1"),
                        ]
                        for j in range(QG):
                            kt_idx = g * QG + j
                            for h in range(2):
                                hb = h * 64
                                nc.tensor.matmul(
                                    sc[h][:, j * 512 : j * 512 + 512],
                                    ktr[
                                        hb : hb + 64,
                                        hp,
                                        kt_idx * 128 : kt_idx * 128 + 128,
                                    ],
                                    qtr[hb : hb + 64, hp, qs],
                                    start=True,
                                    stop=True,
                                )
                        pt = [
                            ptpool.tile([128, 1024], BF16, tag="pt", name="pt0"),
                            ptpool.tile([128, 1024], BF16, tag="pt", name="pt1"),
                        ]
                        for h in range(2):
                            nc.scalar.activation(pt[h], sc[h], AF.Exp, scale=0.125)
                        for j in range(QG):
                            kt_idx = g * QG + j
                            for h in range(2):
                                nc.tensor.matmul(
                                    pv[h][0 : HD + 1, :],
                                    v_sb[:, kt_idx, hp * 2 + h, 0 : HD + 1],
                                    pt[h][:, j * 512 : j * 512 + 512],
                                    start=(kt_idx == 0),
                                    stop=(kt_idx == KT - 1),
                                    skip_group_check=True,
                                )
                    # normalize: att[h-rows, hp, qs] = pv[0:64] * (1/Z bcast)
                    for h in range(2):
                        hb = h * 64
                        o_sb = npool.tile([128, 512], BF16, tag="osb")
                        nc.vector.tensor_copy(o_sb[hb : hb + 64, :], pv[h][0:HD, :])
                        zbf = npool.tile([128, 512], BF16, tag="zbf")
                        with nc.allow_low_precision(
                            reason="bf16 1/Z is within the error budget"
                        ):
                            nc.vector.reciprocal(
                                zbf[HD : HD + 1, :], pv[h][HD : HD + 1, :]
                            )
                        zb = ps_zb.tile([128, 512], F32, tag="zb")
                        nc.tensor.matmul(
                            zb[hb : hb + 64, :],
                            ones_sb[HD : HD + 1, 0:HD],
                            zbf[HD : HD + 1, :],
                            start=True,
                            stop=True,
                        )
                        nc.vector.tensor_mul(
                            att[hb : hb + 64, hp, qs],
                            o_sb[hb : hb + 64, :],
                            zb[hb : hb + 64, :],
                        )
                # output projection for this 512-token stripe
                for tt in range(4):
                    tok = qt * 512 + tt * 128
                    for nh in range(2):
                        ps = ps_wo.tile([128, 512], F32, tag="wo")
                        for hp in range(2):
                            nc.tensor.matmul(
                                ps,
                                att[:, hp, tok : tok + 128],
                                wo_sb[:, hp, nh * 512 : nh * 512 + 512],
                                start=(hp == 0),
                                stop=(hp == 1),
                            )
                        o_out = opool.tile([128, 512], BF16, tag="oo")
                        nc.vector.tensor_copy(o_out, ps)
                        nc.sync.dma_start(
                            out=out[tok : tok + 128, nh * 512 : nh * 512 + 512],
                            in_=o_out,
                        )

    _split_sync_waits(nc, maxw=1)
    return nc


_NC_CACHE = {}


def _rope_cos_sin(seq):
    inv_freq = 1.0 / (
        ROPE_BASE ** (np.arange(0, HD, 2, dtype=np.float32) / HD)
    )
    pos = np.arange(seq, dtype=np.float32)
    freqs = pos[:, None] * inv_freq[None, :]  # [seq, 32]
    emb = np.concatenate([freqs, freqs], axis=-1)  # [seq, 64]
    return np.cos(emb).astype(np.float32), np.sin(emb).astype(np.float32)


def _bf16(a):
    return np.ascontiguousarray(a).astype(ml_dtypes.bfloat16)


def _make_in_maps(hs, W_qkv, W_o, bq, bk, seq):
    cos, sin = _rope_cos_sin(seq)
    cosT = np.ascontiguousarray(cos.T)
    sinT = np.ascontiguousarray(sin.T)
    in_maps = []
    for core in range(N_CORES):
        bb, g = core // 4, core % 4
        cols = slice(g * DLOC, (g + 1) * DLOC)
        in_maps.append(
            {
                "xT": _bf16(hs[bb].T),
                "wq": _bf16(W_qkv[:, cols]),
                "wk": _bf16(W_qkv[:, 1024:][:, cols]),
                "wv": _bf16(W_qkv[:, 2048:][:, cols]),
                "wo": _bf16(W_o[cols, :]),
                "cosT": cosT,
                "sinT": sinT,
                "bqk": np.stack([bq[cols], bk[cols]]).astype(np.float32),
            }
        )
    return in_maps


def kernel(hidden_states, W_qkv, b_qkv, W_o, b_o):
    hs = np.asarray(hidden_states, dtype=np.float32)
    W_qkv = np.asarray(W_qkv, dtype=np.float32)
    b_qkv = np.asarray(b_qkv, dtype=np.float32)
    W_o = np.asarray(W_o, dtype=np.float32)
    b_o = np.asarray(b_o, dtype=np.float32)
    b, seq, d = hs.shape

    bq, bk, bv = b_qkv[:D], b_qkv[D : 2 * D], b_qkv[2 * D :]
    add_qk_bias = bool(np.any(bq) or np.any(bk))

    key = (seq, add_qk_bias)
    if key not in _NC_CACHE:
        _NC_CACHE[key] = build_attention_nc(seq, add_qk_bias)
    nc = _NC_CACHE[key]

    in_maps = _make_in_maps(hs, W_qkv, W_o, bq, bk, seq)

    res = run_bass_kernel_spmd(nc, in_maps, list(range(N_CORES)))
    parts = [
        res.results[c]["out"].astype(np.float32) for c in range(N_CORES)
    ]
    outv = np.stack(
        [parts[0] + parts[1] + parts[2] + parts[3],
         parts[4] + parts[5] + parts[6] + parts[7]]
    )
    outv += b_o[None, None, :] + (bv @ W_o)[None, None, :]
    return outv.astype(np.float32)
